# revision 2
# baseline (speedup 1.0000x reference)
"""GAT layer (nn_GAT_49589692400146) on 8 TRN2 NeuronCores — v3.

Row-shard over nodes (SPMD). Core c owns output rows r0:r1 (R = N/8).

Math: with z1 = x@(W@a1), z2 = x@(W@a2),
  e_ij = lrelu(z1_i + z2_j),  att = softmax_j(mask * exp(e)).
exp(lrelu(t)) = max(exp(t), exp(0.2 t)); scaling row i by exp(-z1_i - c)
(softmax-invariant) gives
  p_ij = m_ij * max(B_j, G_i * D_j)
with B = exp(z2 - c), G = exp(-0.8 z1 - c1), D = exp(0.2 z2 - c + c1).
Equivalently p_ij = exp(0.8 relu(-(z1+z2)) + z2_j - c) masked, which an
ACT Prelu+Exp pair computes directly from a host tensor with the mask
folded in as a +BIG offset (tiny prelu alpha turns +BIG into -inf logits).

z1/z2/B/D/G and Wh = x@W come from the host (O(N F^2) work). Per j-tile
(e^T layout [j, i]) one of four lanes produces p:
  A  fp16: DVE ts-4x pass1, DVE tt-2x mask (fp16 mask), fp16 matmul
  B  fp8:  DVE ts-4x pass1, DVE tt-1x mask (fp8 mask), DoubleRow matmul
  P  fp8:  DVE ts-4x pass1, Pool tt mask (fp8 mask), DoubleRow matmul
  C  fp8:  ACT Prelu + ACT Exp from z1m host tensor, DoubleRow matmul
PE: acc[i] += p^T @ [Wh | 1]; P3: h = numer/den, he = elu(h),
hc = he . fcW_top, s = column-sum(he).
Host: out = concat(hc) + (sum_c s_c) @ fcW_bot + fcb.
"""

import os
import numpy as np
import ml_dtypes

import concourse.bacc as bacc
import concourse.tile as tile
import concourse.mybir as mybir
from concourse import bass_utils

F32 = mybir.dt.float32
F16 = mybir.dt.float16
F8 = mybir.dt.float8e4
ALU = mybir.AluOpType
AF = mybir.ActivationFunctionType
PM = mybir.MatmulPerfMode

NP_F8 = ml_dtypes.float8_e4m3

NCORES = 8
NF = 512
NH = 256
PMAX = 160.0     # target max of p (fp8e4m3 max finite = 240)
MBIG = 45000.0   # mask offset folded into z1m (lane C)
MALPHA = 0.001   # prelu leak: masked logit -> -0.8*MALPHA*MBIG = -36

_BUILD_CACHE = {}

GS = 4  # j-tiles per group


def _lanes(T):
    """Per-tile lane config; groups of GS tiles; even fp8 count per group."""
    s = os.environ.get("GAT_LANES", "")
    if len(s) == T:
        return s
    # fp8 p lanes (B/P/C) fail the accuracy gate on this problem: quantization
    # noise on clustered attention rows sums coherently in the graph-sum term
    # and is amplified ~15x by cancellation in s @ fcW_bot. All-fp16 lanes.
    return "A" * T


def _build(NN, R):
    P = 128
    T = NN // P
    IC = R // P
    NG = T // GS
    assert T % GS == 0 and R % P == 0
    lanes = _lanes(T)
    assert len(lanes) == T
    n_a = lanes.count("A")
    n_f8 = T - n_a
    assert n_f8 % 2 == 0
    for g in range(NG):
        grp = lanes[g * GS:(g + 1) * GS]
        assert (GS - grp.count("A")) % 2 == 0, f"odd fp8 count in group {g}: {grp}"
    n_c = lanes.count("C")
    # packed order per type
    a_ix = {}; f8_ix = {}; c_ix = {}; b_ix = {}
    for t in range(T):
        ln = lanes[t]
        if ln == "A":
            a_ix[t] = len(a_ix)
        else:
            f8_ix[t] = len(f8_ix)
            if ln == "C":
                c_ix[t] = len(c_ix)
            else:
                b_ix[t] = len(b_ix)
    n_b = len(b_ix)

    nc = bacc.Bacc("TRN2", target_bir_lowering=False, debug=False)

    gb_in = nc.dram_tensor("gb_in", [P, R], F16, kind="ExternalInput").ap()
    dbc_in = nc.dram_tensor("dbc_in", [P, 5 * T], F32, kind="ExternalInput").ap()
    fcw_in = nc.dram_tensor("fcw_in", [1, NH], F16, kind="ExternalInput").ap()
    if n_a:
        who16_in = nc.dram_tensor(
            "who16_in", [P, n_a * (NH + 1)], F16, kind="ExternalInput").ap()
        mask16_in = nc.dram_tensor(
            "mask16_in", [P, n_a * R], F16, kind="ExternalInput").ap()
    if n_f8:
        who8_in = nc.dram_tensor(
            "who8_in", [P, n_f8 * (NH + 1)], F8, kind="ExternalInput").ap()
    if n_b:
        mask8_in = nc.dram_tensor(
            "mask8_in", [P, n_b * R], F8, kind="ExternalInput").ap()
    if n_c:
        z1m_in = nc.dram_tensor(
            "z1m_in", [P, n_c * R], F16, kind="ExternalInput").ap()

    hc_out = nc.dram_tensor("hc_out", [R, 1], F32, kind="ExternalOutput").ap()
    sc_out = nc.dram_tensor("sc_out", [1, NH], F32, kind="ExternalOutput").ap()
    dump_pg = os.environ.get("GAT_DUMP_PG") == "1"
    if dump_pg:
        pg_dump = nc.dram_tensor("pg_dump", [P, T * R], F32,
                                 kind="ExternalOutput").ap()

    with tile.TileContext(nc) as tc:
        import contextlib

        with contextlib.ExitStack() as ctx:
            pCst = ctx.enter_context(tc.tile_pool(name="pCst", bufs=1))
            pBig = ctx.enter_context(tc.tile_pool(name="pBig", bufs=1))
            pT = ctx.enter_context(tc.tile_pool(name="pT", bufs=3))
            pP8 = ctx.enter_context(tc.tile_pool(name="pP8", bufs=3))
            pP16 = ctx.enter_context(tc.tile_pool(name="pP16", bufs=3))
            pS = ctx.enter_context(tc.tile_pool(name="pS", bufs=4))
            pP3 = ctx.enter_context(tc.tile_pool(name="pP3", bufs=1))
            psA = ctx.enter_context(tc.tile_pool(name="psA", bufs=1, space="PSUM"))
            psW = ctx.enter_context(tc.tile_pool(name="psW", bufs=2, space="PSUM"))

            # ---- constants first (SP queue) ----
            gb = pCst.tile([P, R], F16, tag="gb")
            nc.sync.dma_start(gb[:], gb_in)
            dbc = pCst.tile([P, 5 * T], F32, tag="dbc")
            nc.sync.dma_start(dbc[:], dbc_in)
            fcwb = pCst.tile([P, NH], F16, tag="fcwb")
            nc.sync.dma_start(fcwb[:], fcw_in.partition_broadcast(P))
            ones_col = pCst.tile([P, 1], F16, tag="ones_col")
            nc.gpsimd.memset(ones_col[:], 1.0)
            hc_sb = pCst.tile([P, IC], F32, tag="hc_sb")
            s_sb = pCst.tile([1, NH], F32, tag="s_sb")

            # ---- whole-tensor streams, interleaved by group-need order ----
            if n_a:
                who16 = pBig.tile([P, n_a * (NH + 1)], F16, tag="who16")
                who16_r = who16[:].rearrange("p (t f) -> p t f", f=NH + 1)
                mask16 = pBig.tile([P, n_a * R], F16, tag="mask16")
            if n_f8:
                who8 = pBig.tile([P, n_f8 * (NH + 1)], F8, tag="who8")
                who8_r = who8[:].rearrange("p (u s f) -> p u s f", s=2, f=NH + 1)
            if n_b:
                mask8 = pBig.tile([P, n_b * R], F8, tag="mask8")
            if n_c:
                z1m = pBig.tile([P, n_c * R], F16, tag="z1m")

            # emit DMAs in ~NCHUNK slabs per tensor, interleaved in the order
            # groups consume them, so every lane starts early.
            NCHUNK = 6
            plans = []      # (first_need_position, dst, src, lo, hi)
            def plan(dst, src, cnt, width, ix_of_tiles):
                if not cnt:
                    return
                per = max(1, (cnt + NCHUNK - 1) // NCHUNK)
                tlist = sorted(ix_of_tiles, key=lambda t: ix_of_tiles[t])
                for s in range(0, cnt, per):
                    lo, hi = s * width, min(cnt, s + per) * width
                    need = tlist[s]  # j-tile of first item in this slab
                    plans.append((need, dst, src, lo, hi))
            if n_a:
                plan(mask16, mask16_in, n_a, R, a_ix)
                plan(who16, who16_in, n_a, NH + 1, a_ix)
            if n_b:
                plan(mask8, mask8_in, n_b, R, b_ix)
            if n_c:
                plan(z1m, z1m_in, n_c, R, c_ix)
            if n_f8:
                plan(who8, who8_in, n_f8, NH + 1, f8_ix)
            for need, dst, src, lo, hi in sorted(plans, key=lambda p: p[0]):
                nc.sync.dma_start(dst[:, lo:hi], src[:, lo:hi])

            acc = [
                psA.tile([P, NH + 1], F32, tag=f"acc{i}", name=f"acc{i}")
                for i in range(IC)
            ]

            first_mm = [True] * IC

            def one_mm(i, pg16_r, pg8_r, kind, k, u, last):
                if kind == 16:
                    nc.tensor.matmul(
                        acc[i][:], pg16_r[:, k, i * P:(i + 1) * P],
                        who16_r[:, u],
                        start=first_mm[i], stop=last,
                    )
                else:
                    nc.tensor.matmul(
                        acc[i][:], pg8_r[:, 2 * k:2 * k + 2, i * P:(i + 1) * P],
                        who8_r[:, u],
                        start=first_mm[i], stop=last,
                        perf_mode=PM.DoubleRow,
                    )
                first_mm[i] = False

            def fp16_mm(pg16_r, k, jt, last):
                for i in range(IC):
                    one_mm(i, pg16_r, None, 16, k, a_ix[jt], last)

            def fp8_mm(pg8_r, v, u, last):
                for i in range(IC):
                    one_mm(i, None, pg8_r, 8, v, u, last)

            DEFER_G = min(2, NG)   # trailing groups emitted i-outer with P3
            mm_defer = []          # (kind, pg_r, k, u)
            for g in range(NG):
                g0 = g * GS
                grp = lanes[g0:g0 + GS]
                ats = [g0 + k for k in range(GS) if grp[k] == "A"]
                f8s = [g0 + k for k in range(GS) if grp[k] != "A"]
                last_g = g >= NG - DEFER_G

                # pass1 targets: shared fp16 t-tile for A/B/P tiles of group
                n1 = len([t for t in ats + f8s if lanes[t] != "C"])
                tm = pT.tile([P, n1 * R], F16, tag="tm", name=f"tm{g}") if n1 else None
                tmap = {}
                k = 0
                for t in [t for t in f8s if lanes[t] != "C"] + ats:
                    tmap[t] = k
                    k += 1

                def pass1(jt):
                    dst = tm[:, tmap[jt] * R:(tmap[jt] + 1) * R]
                    nc.vector.tensor_scalar(
                        dst, gb[:], dbc[:, 5 * jt:5 * jt + 1],
                        dbc[:, 5 * jt + 1:5 * jt + 2], ALU.mult, ALU.max)
                    return dst

                pg8 = pP8.tile([P, len(f8s) * R], F8, tag="pg8",
                               name=f"pg8_{g}") if f8s else None
                pg16 = pP16.tile([P, len(ats) * R], F16, tag="pg16",
                                 name=f"pg16_{g}") if ats else None
                if pg8 is not None:
                    pg8_r = pg8[:].rearrange("p (t r) -> p t r", r=R)
                if pg16 is not None:
                    pg16_r = pg16[:].rearrange("p (t r) -> p t r", r=R)

                # fp8 tiles first (pool lanes early), then A tiles
                for v, jt in enumerate(f8s):
                    ln = lanes[jt]
                    dst = pg8_r[:, v]
                    if ln == "C":
                        ci = c_ix[jt]
                        src = z1m[:, ci * R:(ci + 1) * R]
                        q = pS.tile([P, R], F16, tag="q")
                        nc.scalar.activation(
                            q[:], src, AF.Prelu,
                            bias=dbc[:, 5 * jt + 3:5 * jt + 4], scale=-0.8,
                            alpha=MALPHA)
                        nc.scalar.activation(
                            dst, q[:], AF.Exp,
                            bias=dbc[:, 5 * jt + 4:5 * jt + 5])
                    else:
                        t1 = pass1(jt)
                        bi = b_ix[jt]
                        mk = mask8[:, bi * R:(bi + 1) * R]
                        if ln == "P":
                            nc.gpsimd.tensor_tensor(dst, t1, mk, op=ALU.mult)
                        else:
                            nc.vector.tensor_tensor(dst, t1, mk, op=ALU.mult)
                    if v % 2 == 1:
                        u = f8_ix[f8s[v - 1]] // 2
                        assert f8_ix[f8s[v]] == f8_ix[f8s[v - 1]] + 1
                        if last_g:
                            mm_defer.append((8, pg8_r, v // 2, u))
                        else:
                            fp8_mm(pg8_r, v // 2, u, False)

                if ats:
                    for k, jt in enumerate(ats):
                        pass1(jt)
                    # batched 2x mask multiply over contiguous A block,
                    # column-split between Pool (front) and DVE (rest)
                    a0 = tmap[ats[0]]
                    assert all(tmap[jt] == a0 + k for k, jt in enumerate(ats))
                    m0 = a_ix[ats[0]] * R
                    WA = len(ats) * R
                    WP = min(WA, (int(R * 1.45) // 2) * 2)
                    if WP:
                        nc.gpsimd.tensor_tensor(
                            pg16[:, 0:WP], tm[:, a0 * R:a0 * R + WP],
                            mask16[:, m0:m0 + WP], op=ALU.mult)
                    if WP < WA:
                        nc.vector.tensor_tensor(
                            pg16[:, WP:], tm[:, a0 * R + WP:a0 * R + WA],
                            mask16[:, m0 + WP:m0 + WA], op=ALU.mult)
                    for k, jt in enumerate(ats):
                        if last_g:
                            mm_defer.append((16, pg16_r, k, a_ix[jt]))
                        else:
                            fp16_mm(pg16_r, k, jt, False)

                if dump_pg:
                    for v, jt in enumerate(f8s):
                        dcp = pS.tile([P, R], F32, tag="dcp")
                        nc.vector.tensor_copy(dcp[:], pg8_r[:, v])
                        nc.sync.dma_start(pg_dump[:, jt * R:(jt + 1) * R], dcp[:])
                    for k, jt in enumerate(ats):
                        dcp = pS.tile([P, R], F32, tag="dcp")
                        nc.vector.tensor_copy(dcp[:], pg16_r[:, k])
                        nc.sync.dma_start(pg_dump[:, jt * R:(jt + 1) * R], dcp[:])

            # ---- tail: deferred matmuls i-outer, P3 interleaved per bank ----
            # h = numer/den; ex = exp(h) and rl = relu(h) fused from PSUM;
            # he = elu(h) = min(ex - 1, rl)
            sacc = psW.tile([1, NH], F32, tag="work")
            for i in range(IC):
                for n, (kind, pg_r, k, u) in enumerate(mm_defer):
                    one_mm(i, pg_r, pg_r, kind, k, u, n == len(mm_defer) - 1)
                rec = pP3.tile([P, 1], F32, tag=f"rec{i}")
                nc.vector.reciprocal(rec[:], acc[i][:, NH:NH + 1])
                ex = pP3.tile([P, NH], F16, tag=f"ex{i}")
                nc.scalar.activation(ex[:], acc[i][:, 0:NH], AF.Exp,
                                     scale=rec[:])
                rl = pP3.tile([P, NH], F16, tag=f"rl{i}")
                nc.vector.tensor_scalar(rl[:], acc[i][:, 0:NH], rec[:], 0.0,
                                        ALU.mult, ALU.max)
                he = pP3.tile([P, NH], F16, tag=f"he{i}")
                nc.vector.scalar_tensor_tensor(
                    he[:], ex[:], -1.0, rl[:], ALU.add, ALU.min)
                nc.tensor.matmul(
                    sacc[:], ones_col[:], he[:],
                    start=(i == 0), stop=(i == IC - 1),
                )
                hw = pP3.tile([P, NH], F16, tag=f"hw{i}")
                nc.vector.scalar_tensor_tensor(
                    hw[:], he[:], 1.0, fcwb[:],
                    ALU.mult, ALU.mult, accum_out=hc_sb[:, i:i + 1]
                )

            nc.vector.tensor_copy(s_sb[:], sacc[:])
            nc.sync.dma_start(sc_out, s_sb[:])
            nc.sync.dma_start(
                hc_out.rearrange("(a p) o -> p (a o)", p=P), hc_sb[:]
            )

    nc.compile()
    return nc


def _get_module(NN, R):
    key = (NN, R, os.environ.get("GAT_LANES", ""))
    if key not in _BUILD_CACHE:
        _BUILD_CACHE[key] = _build(NN, R)
    return _BUILD_CACHE[key]


def _make_in_maps(x, adj, W, a, fcW, n_cores=NCORES):
    NN = x.shape[0]
    R = NN // n_cores
    P = 128
    T = NN // P
    lanes = _lanes(T)

    x64 = x.astype(np.float64)
    W64 = W.astype(np.float64)
    a64 = a.astype(np.float64)[:, 0]
    z1 = x64 @ (W64 @ a64[:NH])
    z2 = x64 @ (W64 @ a64[NH:])

    c = max(z2.max(), -0.8 * z1.min() + 0.2 * z2.max()) - np.log(PMAX)
    c1 = -0.8 * np.median(z1)
    G = np.exp(-0.8 * z1 - c1).astype(np.float16)          # [N] per-i
    D = np.exp(0.2 * z2 - c + c1).astype(np.float32)       # [N] per-j
    B = np.exp(z2 - c).astype(np.float32)                  # [N] per-j

    dbc = np.empty((P, 5 * T), np.float32)
    dbc[:, 0::5] = D.reshape(T, P).T
    dbc[:, 1::5] = B.reshape(T, P).T
    dbc[:, 2::5] = -B.reshape(T, P).T
    dbc[:, 3::5] = (-0.8 * z2).reshape(T, P).T.astype(np.float32)
    dbc[:, 4::5] = (z2 - c).reshape(T, P).T.astype(np.float32)

    Wh = (x @ W).astype(np.float32)
    who = np.concatenate([Wh, np.ones((NN, 1), np.float32)], axis=1)  # [N,NH+1]
    who_t = who.reshape(T, P, NH + 1)

    a_tiles = [t for t in range(T) if lanes[t] == "A"]
    f8_tiles = []
    for g in range(T // GS):
        f8_tiles += [t for t in range(g * GS, (g + 1) * GS) if lanes[t] != "A"]
    b_tiles = [t for t in f8_tiles if lanes[t] != "C"]
    c_tiles = [t for t in f8_tiles if lanes[t] == "C"]

    im_base = {"dbc_in": dbc, "fcw_in": fcW[:NH, 0].astype(np.float16)[None, :]}
    if a_tiles:
        im_base["who16_in"] = np.ascontiguousarray(
            who_t[a_tiles].transpose(1, 0, 2)).astype(np.float16).reshape(P, -1)
    if f8_tiles:
        n8 = len(f8_tiles)
        im_base["who8_in"] = np.ascontiguousarray(
            who_t[f8_tiles].reshape(n8 // 2, 2, P, NH + 1).transpose(2, 0, 1, 3)
        ).astype(NP_F8).reshape(P, -1)

    # mask in e^T layout per tile: m[p, i] = (adj[r0+i, t*128+p] > 0)
    mT = np.ascontiguousarray((adj > 0).T.reshape(T, P, NN).transpose(1, 0, 2))

    in_maps = []
    for cix in range(n_cores):
        r0, r1 = cix * R, (cix + 1) * R
        m_c = mT[:, :, r0:r1]           # [P, T, R] bool-ish uint8
        im = dict(im_base)
        im["gb_in"] = np.ascontiguousarray(
            np.broadcast_to(G[r0:r1][None, :], (P, R)))
        if a_tiles:
            im["mask16_in"] = np.ascontiguousarray(
                m_c[:, a_tiles]).astype(np.float16).reshape(P, -1)
        if b_tiles:
            im["mask8_in"] = np.ascontiguousarray(
                m_c[:, b_tiles]).astype(NP_F8).reshape(P, -1)
        if c_tiles:
            z1loc = z1[r0:r1].astype(np.float32)
            zm = (z1loc[None, None, :]
                  + MBIG * (1.0 - m_c[:, c_tiles].astype(np.float32)))
            im["z1m_in"] = zm.astype(np.float16).reshape(P, -1)
        in_maps.append(im)
    return in_maps


def _run_sharded(x, adj, W, a, fcW, fcb, n_cores=NCORES, **run_kwargs):
    NN = x.shape[0]
    R = NN // n_cores
    nc = _get_module(NN, R)
    in_maps = _make_in_maps(x, adj, W, a, fcW, n_cores)

    res = bass_utils.run_bass_kernel_spmd(
        nc, in_maps, core_ids=list(range(n_cores)), **run_kwargs
    )

    hc = np.concatenate([res.results[c]["hc_out"] for c in range(n_cores)], axis=0)
    s = np.sum([res.results[c]["sc_out"] for c in range(n_cores)], axis=0)[0]
    const = s.astype(np.float64) @ fcW[NH:, 0].astype(np.float64) + float(fcb[0])
    out = hc + np.float32(const)
    return out.astype(np.float32), res


def kernel(x, adj, W, a, fcW, fcb):
    out, _ = _run_sharded(
        np.asarray(x), np.asarray(adj), np.asarray(W),
        np.asarray(a), np.asarray(fcW), np.asarray(fcb),
    )
    return out


# revision 3
# speedup vs baseline: 1.0181x; 1.0181x over previous
"""GAT layer (nn_GAT_49589692400146) on 8 TRN2 NeuronCores — v3.

Row-shard over nodes (SPMD). Core c owns output rows r0:r1 (R = N/8).

Math: with z1 = x@(W@a1), z2 = x@(W@a2),
  e_ij = lrelu(z1_i + z2_j),  att = softmax_j(mask * exp(e)).
exp(lrelu(t)) = max(exp(t), exp(0.2 t)); scaling row i by exp(-z1_i - c)
(softmax-invariant) gives
  p_ij = m_ij * max(B_j, G_i * D_j)
with B = exp(z2 - c), G = exp(-0.8 z1 - c1), D = exp(0.2 z2 - c + c1).
Equivalently p_ij = exp(0.8 relu(-(z1+z2)) + z2_j - c) masked, which an
ACT Prelu+Exp pair computes directly from a host tensor with the mask
folded in as a +BIG offset (tiny prelu alpha turns +BIG into -inf logits).

z1/z2/B/D/G and Wh = x@W come from the host (O(N F^2) work). Per j-tile
(e^T layout [j, i]) one of four lanes produces p:
  A  fp16: DVE ts-4x pass1, DVE tt-2x mask (fp16 mask), fp16 matmul
  B  fp8:  DVE ts-4x pass1, DVE tt-1x mask (fp8 mask), DoubleRow matmul
  P  fp8:  DVE ts-4x pass1, Pool tt mask (fp8 mask), DoubleRow matmul
  C  fp8:  ACT Prelu + ACT Exp from z1m host tensor, DoubleRow matmul
PE: acc[i] += p^T @ [Wh | 1]; P3: h = numer/den, he = elu(h),
hc = he . fcW_top, s = column-sum(he).
Host: out = concat(hc) + (sum_c s_c) @ fcW_bot + fcb.
"""

import os
import numpy as np
import ml_dtypes

import concourse.bacc as bacc
import concourse.tile as tile
import concourse.mybir as mybir
from concourse import bass_utils

F32 = mybir.dt.float32
F16 = mybir.dt.float16
F8 = mybir.dt.float8e4
ALU = mybir.AluOpType
AF = mybir.ActivationFunctionType
PM = mybir.MatmulPerfMode

NP_F8 = ml_dtypes.float8_e4m3

NCORES = 8
NF = 512
NH = 256
PMAX = 160.0     # target max of p (fp8e4m3 max finite = 240)
MBIG = 45000.0   # mask offset folded into z1m (lane C)
MALPHA = 0.001   # prelu leak: masked logit -> -0.8*MALPHA*MBIG = -36

_BUILD_CACHE = {}

GS = 4  # j-tiles per group


def _lanes(T):
    """Per-tile lane config; groups of GS tiles; even fp8 count per group."""
    s = os.environ.get("GAT_LANES", "")
    if len(s) == T:
        return s
    # fp8 p lanes (B/P/C) fail the accuracy gate on this problem: quantization
    # noise on clustered attention rows sums coherently in the graph-sum term
    # and is amplified ~15x by cancellation in s @ fcW_bot.  p stays fp16;
    # lane Q uses an fp8 {0,1} mask (exact) to halve mask DMA, multiplied on
    # Pool/DVE at 1x.
    return "AAQQ" * (T // 4)


def _build(NN, R):
    P = 128
    T = NN // P
    IC = R // P
    NG = T // GS
    assert T % GS == 0 and R % P == 0
    lanes = _lanes(T)
    assert len(lanes) == T
    n_a = lanes.count("A") + lanes.count("Q")   # fp16-p tiles (A and Q)
    n_f8 = T - n_a
    assert n_f8 % 2 == 0
    for g in range(NG):
        grp = lanes[g * GS:(g + 1) * GS]
        nf8 = GS - grp.count("A") - grp.count("Q")
        assert nf8 % 2 == 0, f"odd fp8 count in group {g}: {grp}"
    n_c = lanes.count("C")
    # packed order per type
    a_ix = {}; f8_ix = {}; c_ix = {}; b_ix = {}
    m16_ix = {}; q_ix = {}
    for t in range(T):
        ln = lanes[t]
        if ln in "AQ":
            a_ix[t] = len(a_ix)
            if ln == "A":
                m16_ix[t] = len(m16_ix)
            else:
                q_ix[t] = len(q_ix)
        else:
            f8_ix[t] = len(f8_ix)
            if ln == "C":
                c_ix[t] = len(c_ix)
            else:
                b_ix[t] = len(b_ix)
    n_b = len(b_ix)
    n_m16 = len(m16_ix)
    n_q = len(q_ix)

    nc = bacc.Bacc("TRN2", target_bir_lowering=False, debug=False)

    gb_in = nc.dram_tensor("gb_in", [P, R], F16, kind="ExternalInput").ap()
    dbc_in = nc.dram_tensor("dbc_in", [P, 5 * T], F32, kind="ExternalInput").ap()
    fcw_in = nc.dram_tensor("fcw_in", [1, NH], F16, kind="ExternalInput").ap()
    if n_a:
        who16_in = nc.dram_tensor(
            "who16_in", [P, n_a * (NH + 1)], F16, kind="ExternalInput").ap()
        mask16_in = nc.dram_tensor(
            "mask16_in", [P, max(1, n_m16) * R], F16, kind="ExternalInput").ap()
        maskq_in = nc.dram_tensor(
            "maskq_in", [P, max(1, n_q) * R], F8, kind="ExternalInput").ap()
    if n_f8:
        who8_in = nc.dram_tensor(
            "who8_in", [P, n_f8 * (NH + 1)], F8, kind="ExternalInput").ap()
    if n_b:
        mask8_in = nc.dram_tensor(
            "mask8_in", [P, n_b * R], F8, kind="ExternalInput").ap()
    if n_c:
        z1m_in = nc.dram_tensor(
            "z1m_in", [P, n_c * R], F16, kind="ExternalInput").ap()

    hc_out = nc.dram_tensor("hc_out", [R, 1], F32, kind="ExternalOutput").ap()
    sc_out = nc.dram_tensor("sc_out", [1, NH], F32, kind="ExternalOutput").ap()
    dump_pg = os.environ.get("GAT_DUMP_PG") == "1"
    if dump_pg:
        pg_dump = nc.dram_tensor("pg_dump", [P, T * R], F32,
                                 kind="ExternalOutput").ap()

    with tile.TileContext(nc) as tc:
        import contextlib

        with contextlib.ExitStack() as ctx:
            pCst = ctx.enter_context(tc.tile_pool(name="pCst", bufs=1))
            pBig = ctx.enter_context(tc.tile_pool(name="pBig", bufs=1))
            pT = ctx.enter_context(tc.tile_pool(name="pT", bufs=4))
            pP8 = ctx.enter_context(tc.tile_pool(name="pP8", bufs=3))
            pP16 = ctx.enter_context(tc.tile_pool(name="pP16", bufs=4))
            pS = ctx.enter_context(tc.tile_pool(name="pS", bufs=4))
            pP3 = ctx.enter_context(tc.tile_pool(name="pP3", bufs=1))
            psA = ctx.enter_context(tc.tile_pool(name="psA", bufs=1, space="PSUM"))
            psW = ctx.enter_context(tc.tile_pool(name="psW", bufs=2, space="PSUM"))

            # ---- constants first (SP queue) ----
            gb = pCst.tile([P, R], F16, tag="gb")
            nc.sync.dma_start(gb[:], gb_in)
            dbc = pCst.tile([P, 5 * T], F32, tag="dbc")
            nc.sync.dma_start(dbc[:], dbc_in)
            fcwb = pCst.tile([P, NH], F16, tag="fcwb")
            nc.sync.dma_start(fcwb[:], fcw_in.partition_broadcast(P))
            ones_col = pCst.tile([P, 1], F16, tag="ones_col")
            nc.gpsimd.memset(ones_col[:], 1.0)
            hc_sb = pCst.tile([P, IC], F32, tag="hc_sb")
            s_sb = pCst.tile([1, NH], F32, tag="s_sb")

            # ---- whole-tensor streams, interleaved by group-need order ----
            if n_a:
                who16 = pBig.tile([P, n_a * (NH + 1)], F16, tag="who16")
                who16_r = who16[:].rearrange("p (t f) -> p t f", f=NH + 1)
                mask16 = pBig.tile([P, max(1, n_m16) * R], F16, tag="mask16")
                maskq = pBig.tile([P, max(1, n_q) * R], F8, tag="maskq")
            if n_f8:
                who8 = pBig.tile([P, n_f8 * (NH + 1)], F8, tag="who8")
                who8_r = who8[:].rearrange("p (u s f) -> p u s f", s=2, f=NH + 1)
            if n_b:
                mask8 = pBig.tile([P, n_b * R], F8, tag="mask8")
            if n_c:
                z1m = pBig.tile([P, n_c * R], F16, tag="z1m")

            # emit DMAs in ~NCHUNK slabs per tensor, interleaved in the order
            # groups consume them, so every lane starts early.
            NCHUNK = 12
            plans = []      # (first_need_position, dst, src, lo, hi)
            def plan(dst, src, cnt, width, ix_of_tiles):
                if not cnt:
                    return
                per = max(1, (cnt + NCHUNK - 1) // NCHUNK)
                tlist = sorted(ix_of_tiles, key=lambda t: ix_of_tiles[t])
                for s in range(0, cnt, per):
                    lo, hi = s * width, min(cnt, s + per) * width
                    need = tlist[s]  # j-tile of first item in this slab
                    plans.append((need, dst, src, lo, hi))
            if n_m16:
                plan(mask16, mask16_in, n_m16, R, m16_ix)
            if n_q:
                plan(maskq, maskq_in, n_q, R, q_ix)
            if n_a:
                plan(who16, who16_in, n_a, NH + 1, a_ix)
            if n_b:
                plan(mask8, mask8_in, n_b, R, b_ix)
            if n_c:
                plan(z1m, z1m_in, n_c, R, c_ix)
            if n_f8:
                plan(who8, who8_in, n_f8, NH + 1, f8_ix)
            for need, dst, src, lo, hi in sorted(plans, key=lambda p: p[0]):
                nc.sync.dma_start(dst[:, lo:hi], src[:, lo:hi])

            acc = [
                psA.tile([P, NH + 1], F32, tag=f"acc{i}", name=f"acc{i}")
                for i in range(IC)
            ]

            first_mm = [True] * IC

            def one_mm(i, pg16_r, pg8_r, kind, k, u, last):
                if kind == 16:
                    nc.tensor.matmul(
                        acc[i][:], pg16_r[:, k, i * P:(i + 1) * P],
                        who16_r[:, u],
                        start=first_mm[i], stop=last,
                    )
                else:
                    nc.tensor.matmul(
                        acc[i][:], pg8_r[:, 2 * k:2 * k + 2, i * P:(i + 1) * P],
                        who8_r[:, u],
                        start=first_mm[i], stop=last,
                        perf_mode=PM.DoubleRow,
                    )
                first_mm[i] = False

            def fp16_mm(pg16_r, k, jt, last):
                for i in range(IC):
                    one_mm(i, pg16_r, None, 16, k, a_ix[jt], last)

            def fp8_mm(pg8_r, v, u, last):
                for i in range(IC):
                    one_mm(i, None, pg8_r, 8, v, u, last)

            DEFER_G = min(3, NG)   # trailing groups emitted i-outer with P3
            mm_defer = []          # (kind, pg_r, k, u)
            for g in range(NG):
                g0 = g * GS
                grp = lanes[g0:g0 + GS]
                ats = [g0 + k for k in range(GS) if grp[k] in "AQ"]
                f8s = [g0 + k for k in range(GS) if grp[k] not in "AQ"]
                last_g = g >= NG - DEFER_G

                # pass1 targets: shared fp16 t-tile for A/B/P tiles of group
                n1 = len([t for t in ats + f8s if lanes[t] != "C"])
                tm = pT.tile([P, n1 * R], F16, tag="tm", name=f"tm{g}") if n1 else None
                tmap = {}
                k = 0
                for t in [t for t in f8s if lanes[t] != "C"] + ats:
                    tmap[t] = k
                    k += 1

                def pass1(jt):
                    dst = tm[:, tmap[jt] * R:(tmap[jt] + 1) * R]
                    nc.vector.tensor_scalar(
                        dst, gb[:], dbc[:, 5 * jt:5 * jt + 1],
                        dbc[:, 5 * jt + 1:5 * jt + 2], ALU.mult, ALU.max)
                    return dst

                pg8 = pP8.tile([P, len(f8s) * R], F8, tag="pg8",
                               name=f"pg8_{g}") if f8s else None
                pg16 = pP16.tile([P, len(ats) * R], F16, tag="pg16",
                                 name=f"pg16_{g}") if ats else None
                if pg8 is not None:
                    pg8_r = pg8[:].rearrange("p (t r) -> p t r", r=R)
                if pg16 is not None:
                    pg16_r = pg16[:].rearrange("p (t r) -> p t r", r=R)

                # fp8 tiles first (pool lanes early), then A tiles
                for v, jt in enumerate(f8s):
                    ln = lanes[jt]
                    dst = pg8_r[:, v]
                    if ln == "C":
                        ci = c_ix[jt]
                        src = z1m[:, ci * R:(ci + 1) * R]
                        q = pS.tile([P, R], F16, tag="q")
                        nc.scalar.activation(
                            q[:], src, AF.Prelu,
                            bias=dbc[:, 5 * jt + 3:5 * jt + 4], scale=-0.8,
                            alpha=MALPHA)
                        nc.scalar.activation(
                            dst, q[:], AF.Exp,
                            bias=dbc[:, 5 * jt + 4:5 * jt + 5])
                    else:
                        t1 = pass1(jt)
                        bi = b_ix[jt]
                        mk = mask8[:, bi * R:(bi + 1) * R]
                        if ln == "P":
                            nc.gpsimd.tensor_tensor(dst, t1, mk, op=ALU.mult)
                        else:
                            nc.vector.tensor_tensor(dst, t1, mk, op=ALU.mult)
                    if v % 2 == 1:
                        u = f8_ix[f8s[v - 1]] // 2
                        assert f8_ix[f8s[v]] == f8_ix[f8s[v - 1]] + 1
                        if last_g:
                            mm_defer.append((8, pg8_r, v // 2, u))
                        else:
                            fp8_mm(pg8_r, v // 2, u, False)

                if ats:
                    for k, jt in enumerate(ats):
                        pass1(jt)
                    a0 = tmap[ats[0]]
                    assert all(tmap[jt] == a0 + k for k, jt in enumerate(ats))
                    # A tiles: one batched DVE 2x multiply (fp16 mask);
                    # Q tiles: per-tile 1x multiply (fp8 mask) mostly on Pool
                    a_sub = [jt for jt in ats if lanes[jt] == "A"]
                    if a_sub:
                        k0 = tmap[a_sub[0]] - a0
                        assert all(tmap[jt] - a0 == k0 + k
                                   for k, jt in enumerate(a_sub))
                        m0 = m16_ix[a_sub[0]] * R
                        WA = len(a_sub) * R
                        nc.vector.tensor_tensor(
                            pg16[:, k0 * R:k0 * R + WA],
                            tm[:, (a0 + k0) * R:(a0 + k0) * R + WA],
                            mask16[:, m0:m0 + WA], op=ALU.mult)
                    for jt in ats:
                        if lanes[jt] != "Q":
                            continue
                        k = tmap[jt] - a0
                        qm = maskq[:, q_ix[jt] * R:(q_ix[jt] + 1) * R]
                        tsl = tm[:, tmap[jt] * R:(tmap[jt] + 1) * R]
                        dst = pg16[:, k * R:(k + 1) * R]
                        if q_ix[jt] % 5 == 4:
                            nc.vector.tensor_tensor(dst, tsl, qm, op=ALU.mult)
                        else:
                            nc.gpsimd.tensor_tensor(dst, tsl, qm, op=ALU.mult)
                    for k, jt in enumerate(ats):
                        if last_g:
                            mm_defer.append((16, pg16_r, k, a_ix[jt]))
                        else:
                            fp16_mm(pg16_r, k, jt, False)

                if dump_pg:
                    for v, jt in enumerate(f8s):
                        dcp = pS.tile([P, R], F32, tag="dcp")
                        nc.vector.tensor_copy(dcp[:], pg8_r[:, v])
                        nc.sync.dma_start(pg_dump[:, jt * R:(jt + 1) * R], dcp[:])
                    for k, jt in enumerate(ats):
                        dcp = pS.tile([P, R], F32, tag="dcp")
                        nc.vector.tensor_copy(dcp[:], pg16_r[:, k])
                        nc.sync.dma_start(pg_dump[:, jt * R:(jt + 1) * R], dcp[:])

            # ---- tail: deferred matmuls i-outer, P3 interleaved per bank ----
            # h = numer/den; ex = exp(h) and rl = relu(h) fused from PSUM;
            # he = elu(h) = min(ex - 1, rl)
            sacc = psW.tile([1, NH], F32, tag="work")
            for i in range(IC):
                for n, (kind, pg_r, k, u) in enumerate(mm_defer):
                    one_mm(i, pg_r, pg_r, kind, k, u, n == len(mm_defer) - 1)
                rec = pP3.tile([P, 1], F32, tag=f"rec{i}")
                nc.vector.reciprocal(rec[:], acc[i][:, NH:NH + 1])
                ex = pP3.tile([P, NH], F16, tag=f"ex{i}")
                nc.scalar.activation(ex[:], acc[i][:, 0:NH], AF.Exp,
                                     scale=rec[:])
                rl = pP3.tile([P, NH], F16, tag=f"rl{i}")
                nc.vector.tensor_scalar(rl[:], acc[i][:, 0:NH], rec[:], 0.0,
                                        ALU.mult, ALU.max)
                he = pP3.tile([P, NH], F16, tag=f"he{i}")
                nc.vector.scalar_tensor_tensor(
                    he[:], ex[:], -1.0, rl[:], ALU.add, ALU.min)
                nc.tensor.matmul(
                    sacc[:], ones_col[:], he[:],
                    start=(i == 0), stop=(i == IC - 1),
                )
                hw = pP3.tile([P, NH], F16, tag=f"hw{i}")
                nc.vector.scalar_tensor_tensor(
                    hw[:], he[:], 1.0, fcwb[:],
                    ALU.mult, ALU.mult, accum_out=hc_sb[:, i:i + 1]
                )

            nc.vector.tensor_copy(s_sb[:], sacc[:])
            nc.sync.dma_start(sc_out, s_sb[:])
            nc.sync.dma_start(
                hc_out.rearrange("(a p) o -> p (a o)", p=P), hc_sb[:]
            )

    nc.compile()
    return nc


def _get_module(NN, R):
    key = (NN, R, os.environ.get("GAT_LANES", ""))
    if key not in _BUILD_CACHE:
        _BUILD_CACHE[key] = _build(NN, R)
    return _BUILD_CACHE[key]


def _make_in_maps(x, adj, W, a, fcW, n_cores=NCORES):
    NN = x.shape[0]
    R = NN // n_cores
    P = 128
    T = NN // P
    lanes = _lanes(T)

    x64 = x.astype(np.float64)
    W64 = W.astype(np.float64)
    a64 = a.astype(np.float64)[:, 0]
    z1 = x64 @ (W64 @ a64[:NH])
    z2 = x64 @ (W64 @ a64[NH:])

    c = max(z2.max(), -0.8 * z1.min() + 0.2 * z2.max()) - np.log(PMAX)
    c1 = -0.8 * np.median(z1)
    G = np.exp(-0.8 * z1 - c1).astype(np.float16)          # [N] per-i
    D = np.exp(0.2 * z2 - c + c1).astype(np.float32)       # [N] per-j
    B = np.exp(z2 - c).astype(np.float32)                  # [N] per-j

    dbc = np.empty((P, 5 * T), np.float32)
    dbc[:, 0::5] = D.reshape(T, P).T
    dbc[:, 1::5] = B.reshape(T, P).T
    dbc[:, 2::5] = -B.reshape(T, P).T
    dbc[:, 3::5] = (-0.8 * z2).reshape(T, P).T.astype(np.float32)
    dbc[:, 4::5] = (z2 - c).reshape(T, P).T.astype(np.float32)

    Wh = (x @ W).astype(np.float32)
    who = np.concatenate([Wh, np.ones((NN, 1), np.float32)], axis=1)  # [N,NH+1]
    who_t = who.reshape(T, P, NH + 1)

    a_tiles = [t for t in range(T) if lanes[t] in "AQ"]
    m16_tiles = [t for t in range(T) if lanes[t] == "A"]
    q_tiles = [t for t in range(T) if lanes[t] == "Q"]
    f8_tiles = []
    for g in range(T // GS):
        f8_tiles += [t for t in range(g * GS, (g + 1) * GS)
                     if lanes[t] not in "AQ"]
    b_tiles = [t for t in f8_tiles if lanes[t] != "C"]
    c_tiles = [t for t in f8_tiles if lanes[t] == "C"]

    im_base = {"dbc_in": dbc, "fcw_in": fcW[:NH, 0].astype(np.float16)[None, :]}
    if a_tiles:
        im_base["who16_in"] = np.ascontiguousarray(
            who_t[a_tiles].transpose(1, 0, 2)).astype(np.float16).reshape(P, -1)
    if f8_tiles:
        n8 = len(f8_tiles)
        im_base["who8_in"] = np.ascontiguousarray(
            who_t[f8_tiles].reshape(n8 // 2, 2, P, NH + 1).transpose(2, 0, 1, 3)
        ).astype(NP_F8).reshape(P, -1)

    # mask in e^T layout per tile: m[p, i] = (adj[r0+i, t*128+p] > 0)
    mT = np.ascontiguousarray((adj > 0).T.reshape(T, P, NN).transpose(1, 0, 2))

    in_maps = []
    for cix in range(n_cores):
        r0, r1 = cix * R, (cix + 1) * R
        m_c = mT[:, :, r0:r1]           # [P, T, R] bool-ish uint8
        im = dict(im_base)
        im["gb_in"] = np.ascontiguousarray(
            np.broadcast_to(G[r0:r1][None, :], (P, R)))
        if a_tiles:
            im["mask16_in"] = np.ascontiguousarray(
                m_c[:, m16_tiles if m16_tiles else [0]]
            ).astype(np.float16).reshape(P, -1)
            im["maskq_in"] = np.ascontiguousarray(
                m_c[:, q_tiles if q_tiles else [0]]
            ).astype(NP_F8).reshape(P, -1)
        if b_tiles:
            im["mask8_in"] = np.ascontiguousarray(
                m_c[:, b_tiles]).astype(NP_F8).reshape(P, -1)
        if c_tiles:
            z1loc = z1[r0:r1].astype(np.float32)
            zm = (z1loc[None, None, :]
                  + MBIG * (1.0 - m_c[:, c_tiles].astype(np.float32)))
            im["z1m_in"] = zm.astype(np.float16).reshape(P, -1)
        in_maps.append(im)
    return in_maps


def _run_sharded(x, adj, W, a, fcW, fcb, n_cores=NCORES, **run_kwargs):
    NN = x.shape[0]
    R = NN // n_cores
    nc = _get_module(NN, R)
    in_maps = _make_in_maps(x, adj, W, a, fcW, n_cores)

    res = bass_utils.run_bass_kernel_spmd(
        nc, in_maps, core_ids=list(range(n_cores)), **run_kwargs
    )

    hc = np.concatenate([res.results[c]["hc_out"] for c in range(n_cores)], axis=0)
    s = np.sum([res.results[c]["sc_out"] for c in range(n_cores)], axis=0)[0]
    const = s.astype(np.float64) @ fcW[NH:, 0].astype(np.float64) + float(fcb[0])
    out = hc + np.float32(const)
    return out.astype(np.float32), res


def kernel(x, adj, W, a, fcW, fcb):
    out, _ = _run_sharded(
        np.asarray(x), np.asarray(adj), np.asarray(W),
        np.asarray(a), np.asarray(fcW), np.asarray(fcb),
    )
    return out


# revision 4
# speedup vs baseline: 1.0211x; 1.0030x over previous
"""GAT layer (nn_GAT_49589692400146) on 8 TRN2 NeuronCores — v3.

Row-shard over nodes (SPMD). Core c owns output rows r0:r1 (R = N/8).

Math: with z1 = x@(W@a1), z2 = x@(W@a2),
  e_ij = lrelu(z1_i + z2_j),  att = softmax_j(mask * exp(e)).
exp(lrelu(t)) = max(exp(t), exp(0.2 t)); scaling row i by exp(-z1_i - c)
(softmax-invariant) gives
  p_ij = m_ij * max(B_j, G_i * D_j)
with B = exp(z2 - c), G = exp(-0.8 z1 - c1), D = exp(0.2 z2 - c + c1).
Equivalently p_ij = exp(0.8 relu(-(z1+z2)) + z2_j - c) masked, which an
ACT Prelu+Exp pair computes directly from a host tensor with the mask
folded in as a +BIG offset (tiny prelu alpha turns +BIG into -inf logits).

z1/z2/B/D/G and Wh = x@W come from the host (O(N F^2) work). Per j-tile
(e^T layout [j, i]) one of four lanes produces p:
  A  fp16: DVE ts-4x pass1, DVE tt-2x mask (fp16 mask), fp16 matmul
  B  fp8:  DVE ts-4x pass1, DVE tt-1x mask (fp8 mask), DoubleRow matmul
  P  fp8:  DVE ts-4x pass1, Pool tt mask (fp8 mask), DoubleRow matmul
  C  fp8:  ACT Prelu + ACT Exp from z1m host tensor, DoubleRow matmul
PE: acc[i] += p^T @ [Wh | 1]; P3: h = numer/den, he = elu(h),
hc = he . fcW_top, s = column-sum(he).
Host: out = concat(hc) + (sum_c s_c) @ fcW_bot + fcb.
"""

import os
import numpy as np
import ml_dtypes

import concourse.bacc as bacc
import concourse.tile as tile
import concourse.mybir as mybir
from concourse import bass_utils

F32 = mybir.dt.float32
F16 = mybir.dt.float16
F8 = mybir.dt.float8e4
ALU = mybir.AluOpType
AF = mybir.ActivationFunctionType
PM = mybir.MatmulPerfMode

NP_F8 = ml_dtypes.float8_e4m3

NCORES = 8
NF = 512
NH = 256
PMAX = 160.0     # target max of p (fp8e4m3 max finite = 240)
MBIG = 45000.0   # mask offset folded into z1m (lane C)
MALPHA = 0.001   # prelu leak: masked logit -> -0.8*MALPHA*MBIG = -36

_BUILD_CACHE = {}

GS = 4  # j-tiles per group


def _lanes(T):
    """Per-tile lane config; groups of GS tiles; even fp8 count per group."""
    s = os.environ.get("GAT_LANES", "")
    if len(s) == T:
        return s
    # fp8 p lanes (B/P/C) fail the accuracy gate on this problem: quantization
    # noise on clustered attention rows sums coherently in the graph-sum term
    # and is amplified ~15x by cancellation in s @ fcW_bot.  p stays fp16;
    # lane Q uses an fp8 {0,1} mask (exact) to halve mask DMA, multiplied on
    # Pool/DVE at 1x.
    return "AAQQ" * (T // 4)


def _build(NN, R):
    P = 128
    T = NN // P
    IC = R // P
    NG = T // GS
    assert T % GS == 0 and R % P == 0
    lanes = _lanes(T)
    assert len(lanes) == T
    n_a = lanes.count("A") + lanes.count("Q")   # fp16-p tiles (A and Q)
    n_f8 = T - n_a
    assert n_f8 % 2 == 0
    for g in range(NG):
        grp = lanes[g * GS:(g + 1) * GS]
        nf8 = GS - grp.count("A") - grp.count("Q")
        assert nf8 % 2 == 0, f"odd fp8 count in group {g}: {grp}"
    n_c = lanes.count("C")
    # packed order per type
    a_ix = {}; f8_ix = {}; c_ix = {}; b_ix = {}
    m16_ix = {}; q_ix = {}
    for t in range(T):
        ln = lanes[t]
        if ln in "AQ":
            a_ix[t] = len(a_ix)
            if ln == "A":
                m16_ix[t] = len(m16_ix)
            else:
                q_ix[t] = len(q_ix)
        else:
            f8_ix[t] = len(f8_ix)
            if ln == "C":
                c_ix[t] = len(c_ix)
            else:
                b_ix[t] = len(b_ix)
    n_b = len(b_ix)
    n_m16 = len(m16_ix)
    n_q = len(q_ix)

    nc = bacc.Bacc("TRN2", target_bir_lowering=False, debug=False)

    gb_in = nc.dram_tensor("gb_in", [P, R], F16, kind="ExternalInput").ap()
    dbc_in = nc.dram_tensor("dbc_in", [P, 5 * T], F32, kind="ExternalInput").ap()
    fcw_in = nc.dram_tensor("fcw_in", [1, NH], F16, kind="ExternalInput").ap()
    if n_a:
        who16_in = nc.dram_tensor(
            "who16_in", [P, n_a * (NH + 1)], F16, kind="ExternalInput").ap()
        mask16_in = nc.dram_tensor(
            "mask16_in", [P, max(1, n_m16) * R], F16, kind="ExternalInput").ap()
        maskq_in = nc.dram_tensor(
            "maskq_in", [P, max(1, n_q) * R], F8, kind="ExternalInput").ap()
    if n_f8:
        who8_in = nc.dram_tensor(
            "who8_in", [P, n_f8 * (NH + 1)], F8, kind="ExternalInput").ap()
    if n_b:
        mask8_in = nc.dram_tensor(
            "mask8_in", [P, n_b * R], F8, kind="ExternalInput").ap()
    if n_c:
        z1m_in = nc.dram_tensor(
            "z1m_in", [P, n_c * R], F16, kind="ExternalInput").ap()

    hc_out = nc.dram_tensor("hc_out", [R, 1], F32, kind="ExternalOutput").ap()
    sc_out = nc.dram_tensor("sc_out", [1, NH], F32, kind="ExternalOutput").ap()
    dump_pg = os.environ.get("GAT_DUMP_PG") == "1"
    if dump_pg:
        pg_dump = nc.dram_tensor("pg_dump", [P, T * R], F32,
                                 kind="ExternalOutput").ap()

    with tile.TileContext(nc) as tc:
        import contextlib

        with contextlib.ExitStack() as ctx:
            pCst = ctx.enter_context(tc.tile_pool(name="pCst", bufs=1))
            pBig = ctx.enter_context(tc.tile_pool(name="pBig", bufs=1))
            pT = ctx.enter_context(tc.tile_pool(name="pT", bufs=5))
            pP8 = ctx.enter_context(tc.tile_pool(name="pP8", bufs=3))
            pP16 = ctx.enter_context(tc.tile_pool(name="pP16", bufs=6))
            pS = ctx.enter_context(tc.tile_pool(name="pS", bufs=4))
            pP3 = ctx.enter_context(tc.tile_pool(name="pP3", bufs=1))
            psA = ctx.enter_context(tc.tile_pool(name="psA", bufs=1, space="PSUM"))
            psW = ctx.enter_context(tc.tile_pool(name="psW", bufs=2, space="PSUM"))

            # ---- constants first (SP queue) ----
            gb = pCst.tile([P, R], F16, tag="gb")
            nc.sync.dma_start(gb[:], gb_in)
            dbc = pCst.tile([P, 5 * T], F32, tag="dbc")
            nc.sync.dma_start(dbc[:], dbc_in)
            fcwb = pCst.tile([P, NH], F16, tag="fcwb")
            nc.sync.dma_start(fcwb[:], fcw_in.partition_broadcast(P))
            ones_col = pCst.tile([P, 1], F16, tag="ones_col")
            nc.gpsimd.memset(ones_col[:], 1.0)
            hc_sb = pCst.tile([P, IC], F32, tag="hc_sb")
            s_sb = pCst.tile([1, NH], F32, tag="s_sb")

            # ---- whole-tensor streams, interleaved by group-need order ----
            if n_a:
                who16 = pBig.tile([P, n_a * (NH + 1)], F16, tag="who16")
                who16_r = who16[:].rearrange("p (t f) -> p t f", f=NH + 1)
                mask16 = pBig.tile([P, max(1, n_m16) * R], F16, tag="mask16")
                maskq = pBig.tile([P, max(1, n_q) * R], F8, tag="maskq")
            if n_f8:
                who8 = pBig.tile([P, n_f8 * (NH + 1)], F8, tag="who8")
                who8_r = who8[:].rearrange("p (u s f) -> p u s f", s=2, f=NH + 1)
            if n_b:
                mask8 = pBig.tile([P, n_b * R], F8, tag="mask8")
            if n_c:
                z1m = pBig.tile([P, n_c * R], F16, tag="z1m")

            # emit DMAs in ~NCHUNK slabs per tensor, interleaved in the order
            # groups consume them, so every lane starts early.
            NCHUNK = 12
            plans = []      # (first_need_position, dst, src, lo, hi)
            def plan(dst, src, cnt, width, ix_of_tiles):
                if not cnt:
                    return
                per = max(1, (cnt + NCHUNK - 1) // NCHUNK)
                tlist = sorted(ix_of_tiles, key=lambda t: ix_of_tiles[t])
                for s in range(0, cnt, per):
                    lo, hi = s * width, min(cnt, s + per) * width
                    need = tlist[s]  # j-tile of first item in this slab
                    plans.append((need, dst, src, lo, hi))
            if n_m16:
                plan(mask16, mask16_in, n_m16, R, m16_ix)
            if n_q:
                plan(maskq, maskq_in, n_q, R, q_ix)
            if n_a:
                plan(who16, who16_in, n_a, NH + 1, a_ix)
            if n_b:
                plan(mask8, mask8_in, n_b, R, b_ix)
            if n_c:
                plan(z1m, z1m_in, n_c, R, c_ix)
            if n_f8:
                plan(who8, who8_in, n_f8, NH + 1, f8_ix)
            for need, dst, src, lo, hi in sorted(plans, key=lambda p: p[0]):
                nc.sync.dma_start(dst[:, lo:hi], src[:, lo:hi])

            acc = [
                psA.tile([P, NH + 1], F32, tag=f"acc{i}", name=f"acc{i}")
                for i in range(IC)
            ]

            first_mm = [True] * IC

            def one_mm(i, pg16_r, pg8_r, kind, k, u, last):
                if kind == 16:
                    nc.tensor.matmul(
                        acc[i][:], pg16_r[:, k, i * P:(i + 1) * P],
                        who16_r[:, u],
                        start=first_mm[i], stop=last,
                    )
                else:
                    nc.tensor.matmul(
                        acc[i][:], pg8_r[:, 2 * k:2 * k + 2, i * P:(i + 1) * P],
                        who8_r[:, u],
                        start=first_mm[i], stop=last,
                        perf_mode=PM.DoubleRow,
                    )
                first_mm[i] = False

            def fp16_mm(pg16_r, k, jt, last):
                for i in range(IC):
                    one_mm(i, pg16_r, None, 16, k, a_ix[jt], last)

            def fp8_mm(pg8_r, v, u, last):
                for i in range(IC):
                    one_mm(i, None, pg8_r, 8, v, u, last)

            DEFER_G = min(2, NG)   # trailing groups emitted i-outer with P3
            mm_defer = []          # (kind, pg_r, k, u)
            for g in range(NG):
                g0 = g * GS
                grp = lanes[g0:g0 + GS]
                ats = [g0 + k for k in range(GS) if grp[k] in "AQ"]
                f8s = [g0 + k for k in range(GS) if grp[k] not in "AQ"]
                last_g = g >= NG - DEFER_G

                # pass1 targets: shared fp16 t-tile for A/B/P tiles of group
                n1 = len([t for t in ats + f8s if lanes[t] != "C"])
                tm = pT.tile([P, n1 * R], F16, tag="tm", name=f"tm{g}") if n1 else None
                tmap = {}
                k = 0
                for t in [t for t in f8s if lanes[t] != "C"] + ats:
                    tmap[t] = k
                    k += 1

                def pass1(jt):
                    dst = tm[:, tmap[jt] * R:(tmap[jt] + 1) * R]
                    nc.vector.tensor_scalar(
                        dst, gb[:], dbc[:, 5 * jt:5 * jt + 1],
                        dbc[:, 5 * jt + 1:5 * jt + 2], ALU.mult, ALU.max)
                    return dst

                pg8 = pP8.tile([P, len(f8s) * R], F8, tag="pg8",
                               name=f"pg8_{g}") if f8s else None
                pg16 = pP16.tile([P, len(ats) * R], F16, tag="pg16",
                                 name=f"pg16_{g}") if ats else None
                if pg8 is not None:
                    pg8_r = pg8[:].rearrange("p (t r) -> p t r", r=R)
                if pg16 is not None:
                    pg16_r = pg16[:].rearrange("p (t r) -> p t r", r=R)

                # fp8 tiles first (pool lanes early), then A tiles
                for v, jt in enumerate(f8s):
                    ln = lanes[jt]
                    dst = pg8_r[:, v]
                    if ln == "C":
                        ci = c_ix[jt]
                        src = z1m[:, ci * R:(ci + 1) * R]
                        q = pS.tile([P, R], F16, tag="q")
                        nc.scalar.activation(
                            q[:], src, AF.Prelu,
                            bias=dbc[:, 5 * jt + 3:5 * jt + 4], scale=-0.8,
                            alpha=MALPHA)
                        nc.scalar.activation(
                            dst, q[:], AF.Exp,
                            bias=dbc[:, 5 * jt + 4:5 * jt + 5])
                    else:
                        t1 = pass1(jt)
                        bi = b_ix[jt]
                        mk = mask8[:, bi * R:(bi + 1) * R]
                        if ln == "P":
                            nc.gpsimd.tensor_tensor(dst, t1, mk, op=ALU.mult)
                        else:
                            nc.vector.tensor_tensor(dst, t1, mk, op=ALU.mult)
                    if v % 2 == 1:
                        u = f8_ix[f8s[v - 1]] // 2
                        assert f8_ix[f8s[v]] == f8_ix[f8s[v - 1]] + 1
                        if last_g:
                            mm_defer.append((8, pg8_r, v // 2, u))
                        else:
                            fp8_mm(pg8_r, v // 2, u, False)

                if ats:
                    for k, jt in enumerate(ats):
                        pass1(jt)
                    a0 = tmap[ats[0]]
                    assert all(tmap[jt] == a0 + k for k, jt in enumerate(ats))
                    # A tiles: one batched DVE 2x multiply (fp16 mask);
                    # Q tiles: per-tile 1x multiply (fp8 mask) mostly on Pool
                    a_sub = [jt for jt in ats if lanes[jt] == "A"]
                    if a_sub:
                        k0 = tmap[a_sub[0]] - a0
                        assert all(tmap[jt] - a0 == k0 + k
                                   for k, jt in enumerate(a_sub))
                        m0 = m16_ix[a_sub[0]] * R
                        WA = len(a_sub) * R
                        nc.vector.tensor_tensor(
                            pg16[:, k0 * R:k0 * R + WA],
                            tm[:, (a0 + k0) * R:(a0 + k0) * R + WA],
                            mask16[:, m0:m0 + WA], op=ALU.mult)
                    for jt in ats:
                        if lanes[jt] != "Q":
                            continue
                        k = tmap[jt] - a0
                        qm = maskq[:, q_ix[jt] * R:(q_ix[jt] + 1) * R]
                        tsl = tm[:, tmap[jt] * R:(tmap[jt] + 1) * R]
                        dst = pg16[:, k * R:(k + 1) * R]
                        if q_ix[jt] % 5 == 4:
                            nc.vector.tensor_tensor(dst, tsl, qm, op=ALU.mult)
                        else:
                            nc.gpsimd.tensor_tensor(dst, tsl, qm, op=ALU.mult)
                    for k, jt in enumerate(ats):
                        if last_g:
                            mm_defer.append((16, pg16_r, k, a_ix[jt]))
                        else:
                            fp16_mm(pg16_r, k, jt, False)

                if dump_pg:
                    for v, jt in enumerate(f8s):
                        dcp = pS.tile([P, R], F32, tag="dcp")
                        nc.vector.tensor_copy(dcp[:], pg8_r[:, v])
                        nc.sync.dma_start(pg_dump[:, jt * R:(jt + 1) * R], dcp[:])
                    for k, jt in enumerate(ats):
                        dcp = pS.tile([P, R], F32, tag="dcp")
                        nc.vector.tensor_copy(dcp[:], pg16_r[:, k])
                        nc.sync.dma_start(pg_dump[:, jt * R:(jt + 1) * R], dcp[:])

            # ---- tail: deferred matmuls i-outer, P3 interleaved per bank ----
            # h = numer/den; ex = exp(h) and rl = relu(h) fused from PSUM;
            # he = elu(h) = min(ex - 1, rl)
            sacc = psW.tile([1, NH], F32, tag="work")
            for i in range(IC):
                for n, (kind, pg_r, k, u) in enumerate(mm_defer):
                    one_mm(i, pg_r, pg_r, kind, k, u, n == len(mm_defer) - 1)
                rec = pP3.tile([P, 1], F32, tag=f"rec{i}")
                nc.vector.reciprocal(rec[:], acc[i][:, NH:NH + 1])
                ex = pP3.tile([P, NH], F16, tag=f"ex{i}")
                nc.scalar.activation(ex[:], acc[i][:, 0:NH], AF.Exp,
                                     scale=rec[:])
                rl = pP3.tile([P, NH], F16, tag=f"rl{i}")
                nc.scalar.activation(rl[:], acc[i][:, 0:NH], AF.Relu,
                                     scale=rec[:])
                he = pP3.tile([P, NH], F16, tag=f"he{i}")
                nc.vector.scalar_tensor_tensor(
                    he[:], ex[:], -1.0, rl[:], ALU.add, ALU.min)
                nc.tensor.matmul(
                    sacc[:], ones_col[:], he[:],
                    start=(i == 0), stop=(i == IC - 1),
                )
                hw = pP3.tile([P, NH], F16, tag=f"hw{i}")
                nc.vector.scalar_tensor_tensor(
                    hw[:], he[:], 1.0, fcwb[:],
                    ALU.mult, ALU.mult, accum_out=hc_sb[:, i:i + 1]
                )

            nc.vector.tensor_copy(s_sb[:], sacc[:])
            nc.sync.dma_start(sc_out, s_sb[:])
            nc.sync.dma_start(
                hc_out.rearrange("(a p) o -> p (a o)", p=P), hc_sb[:]
            )

    nc.compile()
    return nc


def _get_module(NN, R):
    key = (NN, R, os.environ.get("GAT_LANES", ""))
    if key not in _BUILD_CACHE:
        _BUILD_CACHE[key] = _build(NN, R)
    return _BUILD_CACHE[key]


def _make_in_maps(x, adj, W, a, fcW, n_cores=NCORES):
    NN = x.shape[0]
    R = NN // n_cores
    P = 128
    T = NN // P
    lanes = _lanes(T)

    x64 = x.astype(np.float64)
    W64 = W.astype(np.float64)
    a64 = a.astype(np.float64)[:, 0]
    z1 = x64 @ (W64 @ a64[:NH])
    z2 = x64 @ (W64 @ a64[NH:])

    c = max(z2.max(), -0.8 * z1.min() + 0.2 * z2.max()) - np.log(PMAX)
    c1 = -0.8 * np.median(z1)
    G = np.exp(-0.8 * z1 - c1).astype(np.float16)          # [N] per-i
    D = np.exp(0.2 * z2 - c + c1).astype(np.float32)       # [N] per-j
    B = np.exp(z2 - c).astype(np.float32)                  # [N] per-j

    dbc = np.empty((P, 5 * T), np.float32)
    dbc[:, 0::5] = D.reshape(T, P).T
    dbc[:, 1::5] = B.reshape(T, P).T
    dbc[:, 2::5] = -B.reshape(T, P).T
    dbc[:, 3::5] = (-0.8 * z2).reshape(T, P).T.astype(np.float32)
    dbc[:, 4::5] = (z2 - c).reshape(T, P).T.astype(np.float32)

    Wh = (x @ W).astype(np.float32)
    who = np.concatenate([Wh, np.ones((NN, 1), np.float32)], axis=1)  # [N,NH+1]
    who_t = who.reshape(T, P, NH + 1)

    a_tiles = [t for t in range(T) if lanes[t] in "AQ"]
    m16_tiles = [t for t in range(T) if lanes[t] == "A"]
    q_tiles = [t for t in range(T) if lanes[t] == "Q"]
    f8_tiles = []
    for g in range(T // GS):
        f8_tiles += [t for t in range(g * GS, (g + 1) * GS)
                     if lanes[t] not in "AQ"]
    b_tiles = [t for t in f8_tiles if lanes[t] != "C"]
    c_tiles = [t for t in f8_tiles if lanes[t] == "C"]

    im_base = {"dbc_in": dbc, "fcw_in": fcW[:NH, 0].astype(np.float16)[None, :]}
    if a_tiles:
        im_base["who16_in"] = np.ascontiguousarray(
            who_t[a_tiles].transpose(1, 0, 2)).astype(np.float16).reshape(P, -1)
    if f8_tiles:
        n8 = len(f8_tiles)
        im_base["who8_in"] = np.ascontiguousarray(
            who_t[f8_tiles].reshape(n8 // 2, 2, P, NH + 1).transpose(2, 0, 1, 3)
        ).astype(NP_F8).reshape(P, -1)

    # mask in e^T layout per tile: m[p, i] = (adj[r0+i, t*128+p] > 0)
    mT = np.ascontiguousarray((adj > 0).T.reshape(T, P, NN).transpose(1, 0, 2))

    in_maps = []
    for cix in range(n_cores):
        r0, r1 = cix * R, (cix + 1) * R
        m_c = mT[:, :, r0:r1]           # [P, T, R] bool-ish uint8
        im = dict(im_base)
        im["gb_in"] = np.ascontiguousarray(
            np.broadcast_to(G[r0:r1][None, :], (P, R)))
        if a_tiles:
            im["mask16_in"] = np.ascontiguousarray(
                m_c[:, m16_tiles if m16_tiles else [0]]
            ).astype(np.float16).reshape(P, -1)
            im["maskq_in"] = np.ascontiguousarray(
                m_c[:, q_tiles if q_tiles else [0]]
            ).astype(NP_F8).reshape(P, -1)
        if b_tiles:
            im["mask8_in"] = np.ascontiguousarray(
                m_c[:, b_tiles]).astype(NP_F8).reshape(P, -1)
        if c_tiles:
            z1loc = z1[r0:r1].astype(np.float32)
            zm = (z1loc[None, None, :]
                  + MBIG * (1.0 - m_c[:, c_tiles].astype(np.float32)))
            im["z1m_in"] = zm.astype(np.float16).reshape(P, -1)
        in_maps.append(im)
    return in_maps


def _run_sharded(x, adj, W, a, fcW, fcb, n_cores=NCORES, **run_kwargs):
    NN = x.shape[0]
    R = NN // n_cores
    nc = _get_module(NN, R)
    in_maps = _make_in_maps(x, adj, W, a, fcW, n_cores)

    res = bass_utils.run_bass_kernel_spmd(
        nc, in_maps, core_ids=list(range(n_cores)), **run_kwargs
    )

    hc = np.concatenate([res.results[c]["hc_out"] for c in range(n_cores)], axis=0)
    s = np.sum([res.results[c]["sc_out"] for c in range(n_cores)], axis=0)[0]
    const = s.astype(np.float64) @ fcW[NH:, 0].astype(np.float64) + float(fcb[0])
    out = hc + np.float32(const)
    return out.astype(np.float32), res


def kernel(x, adj, W, a, fcW, fcb):
    out, _ = _run_sharded(
        np.asarray(x), np.asarray(adj), np.asarray(W),
        np.asarray(a), np.asarray(fcW), np.asarray(fcb),
    )
    return out


# revision 5
# speedup vs baseline: 1.0298x; 1.0085x over previous
"""GAT layer (nn_GAT_49589692400146) on 8 TRN2 NeuronCores — v3.

Row-shard over nodes (SPMD). Core c owns output rows r0:r1 (R = N/8).

Math: with z1 = x@(W@a1), z2 = x@(W@a2),
  e_ij = lrelu(z1_i + z2_j),  att = softmax_j(mask * exp(e)).
exp(lrelu(t)) = max(exp(t), exp(0.2 t)); scaling row i by exp(-z1_i - c)
(softmax-invariant) gives
  p_ij = m_ij * max(B_j, G_i * D_j)
with B = exp(z2 - c), G = exp(-0.8 z1 - c1), D = exp(0.2 z2 - c + c1).
Equivalently p_ij = exp(0.8 relu(-(z1+z2)) + z2_j - c) masked, which an
ACT Prelu+Exp pair computes directly from a host tensor with the mask
folded in as a +BIG offset (tiny prelu alpha turns +BIG into -inf logits).

z1/z2/B/D/G and Wh = x@W come from the host (O(N F^2) work). Per j-tile
(e^T layout [j, i]) one of four lanes produces p:
  A  fp16: DVE ts-4x pass1, DVE tt-2x mask (fp16 mask), fp16 matmul
  B  fp8:  DVE ts-4x pass1, DVE tt-1x mask (fp8 mask), DoubleRow matmul
  P  fp8:  DVE ts-4x pass1, Pool tt mask (fp8 mask), DoubleRow matmul
  C  fp8:  ACT Prelu + ACT Exp from z1m host tensor, DoubleRow matmul
PE: acc[i] += p^T @ [Wh | 1]; P3: h = numer/den, he = elu(h),
hc = he . fcW_top, s = column-sum(he).
Host: out = concat(hc) + (sum_c s_c) @ fcW_bot + fcb.
"""

import os
import numpy as np
import ml_dtypes

import concourse.bacc as bacc
import concourse.tile as tile
import concourse.mybir as mybir
from concourse import bass_utils

F32 = mybir.dt.float32
F16 = mybir.dt.float16
F8 = mybir.dt.float8e4
ALU = mybir.AluOpType
AF = mybir.ActivationFunctionType
PM = mybir.MatmulPerfMode

NP_F8 = ml_dtypes.float8_e4m3

NCORES = 8
NF = 512
NH = 256
PMAX = 160.0     # target max of p (fp8e4m3 max finite = 240)
MBIG = 45000.0   # mask offset folded into z1m (lane C)
MALPHA = 0.001   # prelu leak: masked logit -> -0.8*MALPHA*MBIG = -36

_BUILD_CACHE = {}

GS = 4  # j-tiles per group


def _lanes(T):
    """Per-tile lane config; groups of GS tiles; even fp8 count per group."""
    s = os.environ.get("GAT_LANES", "")
    if len(s) == T:
        return s
    # fp8 p lanes (B/P/C) fail the accuracy gate on this problem: quantization
    # noise on clustered attention rows sums coherently in the graph-sum term
    # and is amplified ~15x by cancellation in s @ fcW_bot.  p stays fp16;
    # lane Q uses an fp8 {0,1} mask (exact) to halve mask DMA, multiplied on
    # Pool/DVE at 1x.
    return "AAQQ" * (T // 4)


def _build(NN, R):
    P = 128
    T = NN // P
    IC = R // P
    NG = T // GS
    assert T % GS == 0 and R % P == 0
    lanes = _lanes(T)
    assert len(lanes) == T
    n_a = lanes.count("A") + lanes.count("Q")   # fp16-p tiles (A and Q)
    n_f8 = T - n_a
    assert n_f8 % 2 == 0
    for g in range(NG):
        grp = lanes[g * GS:(g + 1) * GS]
        nf8 = GS - grp.count("A") - grp.count("Q")
        assert nf8 % 2 == 0, f"odd fp8 count in group {g}: {grp}"
    n_c = lanes.count("C")
    # packed order per type
    a_ix = {}; f8_ix = {}; c_ix = {}; b_ix = {}
    m16_ix = {}; q_ix = {}
    for t in range(T):
        ln = lanes[t]
        if ln in "AQ":
            a_ix[t] = len(a_ix)
            if ln == "A":
                m16_ix[t] = len(m16_ix)
            else:
                q_ix[t] = len(q_ix)
        else:
            f8_ix[t] = len(f8_ix)
            if ln == "C":
                c_ix[t] = len(c_ix)
            else:
                b_ix[t] = len(b_ix)
    n_b = len(b_ix)
    n_m16 = len(m16_ix)
    n_q = len(q_ix)

    nc = bacc.Bacc("TRN2", target_bir_lowering=False, debug=False)

    gb_in = nc.dram_tensor("gb_in", [P, R], F16, kind="ExternalInput").ap()
    dbc_in = nc.dram_tensor("dbc_in", [P, 5 * T], F32, kind="ExternalInput").ap()
    fcw_in = nc.dram_tensor("fcw_in", [1, NH], F16, kind="ExternalInput").ap()
    if n_a:
        who16_in = nc.dram_tensor(
            "who16_in", [P, n_a * (NH + 1)], F16, kind="ExternalInput").ap()
        mask16_in = nc.dram_tensor(
            "mask16_in", [P, max(1, n_m16) * R], F16, kind="ExternalInput").ap()
        maskq_in = nc.dram_tensor(
            "maskq_in", [P, max(1, n_q) * R], F8, kind="ExternalInput").ap()
    if n_f8:
        who8_in = nc.dram_tensor(
            "who8_in", [P, n_f8 * (NH + 1)], F8, kind="ExternalInput").ap()
    if n_b:
        mask8_in = nc.dram_tensor(
            "mask8_in", [P, n_b * R], F8, kind="ExternalInput").ap()
    if n_c:
        z1m_in = nc.dram_tensor(
            "z1m_in", [P, n_c * R], F16, kind="ExternalInput").ap()

    hc_out = nc.dram_tensor("hc_out", [R, 1], F32, kind="ExternalOutput").ap()
    sc_out = nc.dram_tensor("sc_out", [1, NH], F32, kind="ExternalOutput").ap()
    dump_pg = os.environ.get("GAT_DUMP_PG") == "1"
    if dump_pg:
        pg_dump = nc.dram_tensor("pg_dump", [P, T * R], F32,
                                 kind="ExternalOutput").ap()

    with tile.TileContext(nc) as tc:
        import contextlib

        with contextlib.ExitStack() as ctx:
            pCst = ctx.enter_context(tc.tile_pool(name="pCst", bufs=1))
            pBig = ctx.enter_context(tc.tile_pool(name="pBig", bufs=1))
            pT = ctx.enter_context(tc.tile_pool(name="pT", bufs=5))
            pP8 = ctx.enter_context(tc.tile_pool(name="pP8", bufs=3))
            pP16 = ctx.enter_context(tc.tile_pool(name="pP16", bufs=6))
            pS = ctx.enter_context(tc.tile_pool(name="pS", bufs=4))
            pP3 = ctx.enter_context(tc.tile_pool(name="pP3", bufs=1))
            psA = ctx.enter_context(tc.tile_pool(name="psA", bufs=1, space="PSUM"))
            psW = ctx.enter_context(tc.tile_pool(name="psW", bufs=2, space="PSUM"))

            # ---- constants first (SP queue) ----
            gb = pCst.tile([P, R], F16, tag="gb")
            nc.sync.dma_start(gb[:], gb_in)
            dbc = pCst.tile([P, 5 * T], F32, tag="dbc")
            nc.sync.dma_start(dbc[:], dbc_in)
            fcwb = pCst.tile([P, NH], F16, tag="fcwb")
            nc.sync.dma_start(fcwb[:], fcw_in.partition_broadcast(P))
            ones_col = pCst.tile([P, 1], F16, tag="ones_col")
            nc.gpsimd.memset(ones_col[:], 1.0)
            hc_sb = pCst.tile([P, IC], F32, tag="hc_sb")
            s_sb = pCst.tile([1, NH], F32, tag="s_sb")

            # ---- whole-tensor streams, interleaved by group-need order ----
            if n_a:
                who16 = pBig.tile([P, n_a * (NH + 1)], F16, tag="who16")
                who16_r = who16[:].rearrange("p (t f) -> p t f", f=NH + 1)
                mask16 = pBig.tile([P, max(1, n_m16) * R], F16, tag="mask16")
                maskq = pBig.tile([P, max(1, n_q) * R], F8, tag="maskq")
            if n_f8:
                who8 = pBig.tile([P, n_f8 * (NH + 1)], F8, tag="who8")
                who8_r = who8[:].rearrange("p (u s f) -> p u s f", s=2, f=NH + 1)
            if n_b:
                mask8 = pBig.tile([P, n_b * R], F8, tag="mask8")
            if n_c:
                z1m = pBig.tile([P, n_c * R], F16, tag="z1m")

            # emit DMAs in ~NCHUNK slabs per tensor, interleaved in the order
            # groups consume them, so every lane starts early.
            NCHUNK = 16
            plans = []      # (first_need_position, dst, src, lo, hi)
            def plan(dst, src, cnt, width, ix_of_tiles):
                if not cnt:
                    return
                per = max(1, (cnt + NCHUNK - 1) // NCHUNK)
                tlist = sorted(ix_of_tiles, key=lambda t: ix_of_tiles[t])
                for s in range(0, cnt, per):
                    lo, hi = s * width, min(cnt, s + per) * width
                    need = tlist[s]  # j-tile of first item in this slab
                    plans.append((need, dst, src, lo, hi))
            if n_m16:
                plan(mask16, mask16_in, n_m16, R, m16_ix)
            if n_q:
                plan(maskq, maskq_in, n_q, R, q_ix)
            if n_a:
                plan(who16, who16_in, n_a, NH + 1, a_ix)
            if n_b:
                plan(mask8, mask8_in, n_b, R, b_ix)
            if n_c:
                plan(z1m, z1m_in, n_c, R, c_ix)
            if n_f8:
                plan(who8, who8_in, n_f8, NH + 1, f8_ix)
            for need, dst, src, lo, hi in sorted(plans, key=lambda p: p[0]):
                nc.sync.dma_start(dst[:, lo:hi], src[:, lo:hi])

            acc = [
                psA.tile([P, NH + 1], F32, tag=f"acc{i}", name=f"acc{i}")
                for i in range(IC)
            ]

            first_mm = [True] * IC

            def one_mm(i, pg16_r, pg8_r, kind, k, u, last):
                if kind == 16:
                    nc.tensor.matmul(
                        acc[i][:], pg16_r[:, k, i * P:(i + 1) * P],
                        who16_r[:, u],
                        start=first_mm[i], stop=last,
                    )
                else:
                    nc.tensor.matmul(
                        acc[i][:], pg8_r[:, 2 * k:2 * k + 2, i * P:(i + 1) * P],
                        who8_r[:, u],
                        start=first_mm[i], stop=last,
                        perf_mode=PM.DoubleRow,
                    )
                first_mm[i] = False

            def fp16_mm(pg16_r, k, jt, last):
                for i in range(IC):
                    one_mm(i, pg16_r, None, 16, k, a_ix[jt], last)

            def fp8_mm(pg8_r, v, u, last):
                for i in range(IC):
                    one_mm(i, None, pg8_r, 8, v, u, last)

            DEFER_G = min(2, NG)   # trailing groups emitted i-outer with P3
            mm_defer = []          # (kind, pg_r, k, u)
            for g in range(NG):
                g0 = g * GS
                grp = lanes[g0:g0 + GS]
                ats = [g0 + k for k in range(GS) if grp[k] in "AQ"]
                f8s = [g0 + k for k in range(GS) if grp[k] not in "AQ"]
                last_g = g >= NG - DEFER_G

                # pass1 targets: shared fp16 t-tile for A/B/P tiles of group
                n1 = len([t for t in ats + f8s if lanes[t] != "C"])
                tm = pT.tile([P, n1 * R], F16, tag="tm", name=f"tm{g}") if n1 else None
                tmap = {}
                k = 0
                for t in [t for t in f8s if lanes[t] != "C"] + ats:
                    tmap[t] = k
                    k += 1

                def pass1(jt):
                    dst = tm[:, tmap[jt] * R:(tmap[jt] + 1) * R]
                    nc.vector.tensor_scalar(
                        dst, gb[:], dbc[:, 5 * jt:5 * jt + 1],
                        dbc[:, 5 * jt + 1:5 * jt + 2], ALU.mult, ALU.max)
                    return dst

                pg8 = pP8.tile([P, len(f8s) * R], F8, tag="pg8",
                               name=f"pg8_{g}") if f8s else None
                pg16 = pP16.tile([P, len(ats) * R], F16, tag="pg16",
                                 name=f"pg16_{g}") if ats else None
                if pg8 is not None:
                    pg8_r = pg8[:].rearrange("p (t r) -> p t r", r=R)
                if pg16 is not None:
                    pg16_r = pg16[:].rearrange("p (t r) -> p t r", r=R)

                # fp8 tiles first (pool lanes early), then A tiles
                for v, jt in enumerate(f8s):
                    ln = lanes[jt]
                    dst = pg8_r[:, v]
                    if ln == "C":
                        ci = c_ix[jt]
                        src = z1m[:, ci * R:(ci + 1) * R]
                        q = pS.tile([P, R], F16, tag="q")
                        nc.scalar.activation(
                            q[:], src, AF.Prelu,
                            bias=dbc[:, 5 * jt + 3:5 * jt + 4], scale=-0.8,
                            alpha=MALPHA)
                        nc.scalar.activation(
                            dst, q[:], AF.Exp,
                            bias=dbc[:, 5 * jt + 4:5 * jt + 5])
                    else:
                        t1 = pass1(jt)
                        bi = b_ix[jt]
                        mk = mask8[:, bi * R:(bi + 1) * R]
                        if ln == "P":
                            nc.gpsimd.tensor_tensor(dst, t1, mk, op=ALU.mult)
                        else:
                            nc.vector.tensor_tensor(dst, t1, mk, op=ALU.mult)
                    if v % 2 == 1:
                        u = f8_ix[f8s[v - 1]] // 2
                        assert f8_ix[f8s[v]] == f8_ix[f8s[v - 1]] + 1
                        if last_g:
                            mm_defer.append((8, pg8_r, v // 2, u))
                        else:
                            fp8_mm(pg8_r, v // 2, u, False)

                if ats:
                    for k, jt in enumerate(ats):
                        pass1(jt)
                    a0 = tmap[ats[0]]
                    assert all(tmap[jt] == a0 + k for k, jt in enumerate(ats))
                    # A tiles: one batched DVE 2x multiply (fp16 mask);
                    # Q tiles: per-tile 1x multiply (fp8 mask) mostly on Pool
                    a_sub = [jt for jt in ats if lanes[jt] == "A"]
                    if a_sub:
                        k0 = tmap[a_sub[0]] - a0
                        assert all(tmap[jt] - a0 == k0 + k
                                   for k, jt in enumerate(a_sub))
                        m0 = m16_ix[a_sub[0]] * R
                        WA = len(a_sub) * R
                        nc.vector.tensor_tensor(
                            pg16[:, k0 * R:k0 * R + WA],
                            tm[:, (a0 + k0) * R:(a0 + k0) * R + WA],
                            mask16[:, m0:m0 + WA], op=ALU.mult)
                    for jt in ats:
                        if lanes[jt] != "Q":
                            continue
                        k = tmap[jt] - a0
                        qm = maskq[:, q_ix[jt] * R:(q_ix[jt] + 1) * R]
                        tsl = tm[:, tmap[jt] * R:(tmap[jt] + 1) * R]
                        dst = pg16[:, k * R:(k + 1) * R]
                        if q_ix[jt] % 5 == 4:
                            nc.vector.tensor_tensor(dst, tsl, qm, op=ALU.mult)
                        else:
                            nc.gpsimd.tensor_tensor(dst, tsl, qm, op=ALU.mult)
                    for k, jt in enumerate(ats):
                        if last_g:
                            mm_defer.append((16, pg16_r, k, a_ix[jt]))
                        else:
                            fp16_mm(pg16_r, k, jt, False)

                if dump_pg:
                    for v, jt in enumerate(f8s):
                        dcp = pS.tile([P, R], F32, tag="dcp")
                        nc.vector.tensor_copy(dcp[:], pg8_r[:, v])
                        nc.sync.dma_start(pg_dump[:, jt * R:(jt + 1) * R], dcp[:])
                    for k, jt in enumerate(ats):
                        dcp = pS.tile([P, R], F32, tag="dcp")
                        nc.vector.tensor_copy(dcp[:], pg16_r[:, k])
                        nc.sync.dma_start(pg_dump[:, jt * R:(jt + 1) * R], dcp[:])

            # ---- tail: deferred matmuls i-outer, P3 interleaved per bank ----
            # h = numer/den; ex = exp(h) and rl = relu(h) fused from PSUM;
            # he = elu(h) = min(ex - 1, rl)
            sacc = psW.tile([1, NH], F32, tag="work")
            for i in range(IC):
                for n, (kind, pg_r, k, u) in enumerate(mm_defer):
                    one_mm(i, pg_r, pg_r, kind, k, u, n == len(mm_defer) - 1)
                rec = pP3.tile([P, 1], F32, tag=f"rec{i}")
                nc.vector.reciprocal(rec[:], acc[i][:, NH:NH + 1])
                ex = pP3.tile([P, NH], F16, tag=f"ex{i}")
                nc.scalar.activation(ex[:], acc[i][:, 0:NH], AF.Exp,
                                     scale=rec[:])
                rl = pP3.tile([P, NH], F16, tag=f"rl{i}")
                nc.scalar.activation(rl[:], acc[i][:, 0:NH], AF.Relu,
                                     scale=rec[:])
                he = pP3.tile([P, NH], F16, tag=f"he{i}")
                nc.vector.scalar_tensor_tensor(
                    he[:], ex[:], -1.0, rl[:], ALU.add, ALU.min)
                nc.tensor.matmul(
                    sacc[:], ones_col[:], he[:],
                    start=(i == 0), stop=(i == IC - 1),
                )
                hw = pP3.tile([P, NH], F16, tag=f"hw{i}")
                nc.vector.scalar_tensor_tensor(
                    hw[:], he[:], 1.0, fcwb[:],
                    ALU.mult, ALU.mult, accum_out=hc_sb[:, i:i + 1]
                )

            nc.vector.tensor_copy(s_sb[:], sacc[:])
            nc.sync.dma_start(sc_out, s_sb[:])
            nc.sync.dma_start(
                hc_out.rearrange("(a p) o -> p (a o)", p=P), hc_sb[:]
            )

    nc.compile()
    return nc


def _get_module(NN, R):
    key = (NN, R, os.environ.get("GAT_LANES", ""))
    if key not in _BUILD_CACHE:
        _BUILD_CACHE[key] = _build(NN, R)
    return _BUILD_CACHE[key]


def _make_in_maps(x, adj, W, a, fcW, n_cores=NCORES):
    NN = x.shape[0]
    R = NN // n_cores
    P = 128
    T = NN // P
    lanes = _lanes(T)

    x64 = x.astype(np.float64)
    W64 = W.astype(np.float64)
    a64 = a.astype(np.float64)[:, 0]
    z1 = x64 @ (W64 @ a64[:NH])
    z2 = x64 @ (W64 @ a64[NH:])

    c = max(z2.max(), -0.8 * z1.min() + 0.2 * z2.max()) - np.log(PMAX)
    c1 = -0.8 * np.median(z1)
    G = np.exp(-0.8 * z1 - c1).astype(np.float16)          # [N] per-i
    D = np.exp(0.2 * z2 - c + c1).astype(np.float32)       # [N] per-j
    B = np.exp(z2 - c).astype(np.float32)                  # [N] per-j

    dbc = np.empty((P, 5 * T), np.float32)
    dbc[:, 0::5] = D.reshape(T, P).T
    dbc[:, 1::5] = B.reshape(T, P).T
    dbc[:, 2::5] = -B.reshape(T, P).T
    dbc[:, 3::5] = (-0.8 * z2).reshape(T, P).T.astype(np.float32)
    dbc[:, 4::5] = (z2 - c).reshape(T, P).T.astype(np.float32)

    Wh = (x @ W).astype(np.float32)
    who = np.concatenate([Wh, np.ones((NN, 1), np.float32)], axis=1)  # [N,NH+1]
    who_t = who.reshape(T, P, NH + 1)

    a_tiles = [t for t in range(T) if lanes[t] in "AQ"]
    m16_tiles = [t for t in range(T) if lanes[t] == "A"]
    q_tiles = [t for t in range(T) if lanes[t] == "Q"]
    f8_tiles = []
    for g in range(T // GS):
        f8_tiles += [t for t in range(g * GS, (g + 1) * GS)
                     if lanes[t] not in "AQ"]
    b_tiles = [t for t in f8_tiles if lanes[t] != "C"]
    c_tiles = [t for t in f8_tiles if lanes[t] == "C"]

    im_base = {"dbc_in": dbc, "fcw_in": fcW[:NH, 0].astype(np.float16)[None, :]}
    if a_tiles:
        im_base["who16_in"] = np.ascontiguousarray(
            who_t[a_tiles].transpose(1, 0, 2)).astype(np.float16).reshape(P, -1)
    if f8_tiles:
        n8 = len(f8_tiles)
        im_base["who8_in"] = np.ascontiguousarray(
            who_t[f8_tiles].reshape(n8 // 2, 2, P, NH + 1).transpose(2, 0, 1, 3)
        ).astype(NP_F8).reshape(P, -1)

    # mask in e^T layout per tile: m[p, i] = (adj[r0+i, t*128+p] > 0)
    mT = np.ascontiguousarray((adj > 0).T.reshape(T, P, NN).transpose(1, 0, 2))

    in_maps = []
    for cix in range(n_cores):
        r0, r1 = cix * R, (cix + 1) * R
        m_c = mT[:, :, r0:r1]           # [P, T, R] bool-ish uint8
        im = dict(im_base)
        im["gb_in"] = np.ascontiguousarray(
            np.broadcast_to(G[r0:r1][None, :], (P, R)))
        if a_tiles:
            im["mask16_in"] = np.ascontiguousarray(
                m_c[:, m16_tiles if m16_tiles else [0]]
            ).astype(np.float16).reshape(P, -1)
            im["maskq_in"] = np.ascontiguousarray(
                m_c[:, q_tiles if q_tiles else [0]]
            ).astype(NP_F8).reshape(P, -1)
        if b_tiles:
            im["mask8_in"] = np.ascontiguousarray(
                m_c[:, b_tiles]).astype(NP_F8).reshape(P, -1)
        if c_tiles:
            z1loc = z1[r0:r1].astype(np.float32)
            zm = (z1loc[None, None, :]
                  + MBIG * (1.0 - m_c[:, c_tiles].astype(np.float32)))
            im["z1m_in"] = zm.astype(np.float16).reshape(P, -1)
        in_maps.append(im)
    return in_maps


def _run_sharded(x, adj, W, a, fcW, fcb, n_cores=NCORES, **run_kwargs):
    NN = x.shape[0]
    R = NN // n_cores
    nc = _get_module(NN, R)
    in_maps = _make_in_maps(x, adj, W, a, fcW, n_cores)

    res = bass_utils.run_bass_kernel_spmd(
        nc, in_maps, core_ids=list(range(n_cores)), **run_kwargs
    )

    hc = np.concatenate([res.results[c]["hc_out"] for c in range(n_cores)], axis=0)
    s = np.sum([res.results[c]["sc_out"] for c in range(n_cores)], axis=0)[0]
    const = s.astype(np.float64) @ fcW[NH:, 0].astype(np.float64) + float(fcb[0])
    out = hc + np.float32(const)
    return out.astype(np.float32), res


def kernel(x, adj, W, a, fcW, fcb):
    out, _ = _run_sharded(
        np.asarray(x), np.asarray(adj), np.asarray(W),
        np.asarray(a), np.asarray(fcW), np.asarray(fcb),
    )
    return out
